# revision 1
# baseline (speedup 1.0000x reference)
"""ControlCPC loss kernel for 8 Trainium2 NeuronCores (Bass/Tile).

Row-sharded over the batch: core c owns rows [128c, 128c+128).

Algebraic reduction of the reference (B=1024, Z=512, A=8, AH=64, ZH=512,
n_neg=100, T=0.1):
  sim[b, j] = z_next[b] . z_next_hat[j]
  u[b]      = z[b] @ W1[:Z] + b1            (shared across shifts)
  g[j]      = relu(actions[j] @ Wa + ba) @ W1[Z:]
  v[b]      = z_next[b] @ W2.T
  neg[b, i] = z[b].z_next[b] + b2.z_next[b]
              + sum_zh relu(u[b,zh] + g[b+i,zh]) * v[b,zh]     i = 1..99
  row stats over x = [sim | neg]: m = rowmax, S = sum exp((x-m)/T),
  diag = sim[b,b] (extracted bit-exact from PSUM), cnt = #{x > diag}.
Host: loss = mean(m/T + ln S - diag/T), acc_k = mean(cnt < k).

v2 engine plan (per shift):
  - PE: two fp8e4 DoubleRow matmuls produce t = u + rot(g, i) in PSUM:
    DR#1 sums the two shifted-identity picks of g0/g1 (one k-pair), DR#2
    adds u via the i=0 identity window (slot1 weight is the zero block).
    0.5 cycles/row -> ~215 ns/shift vs ~900 ns for 3 bf16 matmuls.
  - Shift pairs share a [128, 2*512] PSUM tile (2 banks). Consumers mix
    three paths, LP-balanced across engines:
      D : DVE custom RELU_MUL_REDUCE direct from PSUM      (~750 ns DVE)
      AD: one paired Act relu->bf16 copy (~520/shift), DVE op from SBUF
      AP: same Act copy, Pool STT from SBUF                (~850 ns Pool)
  - Row stats / sim block / diag extraction unchanged from v1.
fp8 t-error ~0.03/elem -> neg-value error ~0.3; top-k boundary margin is
~6.9 and the loss tolerance 2e-2, both orders of magnitude away.
"""

import sys

for _p in ("/opt/trn_rl_repo", "/opt/pypackages"):
    if _p not in sys.path:
        sys.path.insert(0, _p)

import numpy as np
import ml_dtypes

import concourse.bass as bass
import concourse.mybir as mybir
import concourse.tile as tile
from concourse import bacc
from concourse.bass_utils import run_bass_kernel_spmd

f32 = mybir.dt.float32
f32r = mybir.dt.float32r
bf16 = mybir.dt.bfloat16
fp8e4 = mybir.dt.float8e4
AL = mybir.AluOpType
AF = mybir.ActivationFunctionType
PM = mybir.MatmulPerfMode

B, Z, A = 1024, 512, 8
AH, ZH = 64, 512
TEMP = 0.1
NCORES = 8
R = B // NCORES          # 128 rows per core
NSH = 99                 # shifts 1..99
INV_T = 1.0 / TEMP
TOP_K = (1, 3, 10)

_cache = {}


def _register_custom_ops():
    from concourse.dve_ops import DveOp, OPS
    from concourse.dve_spec import Spec, Src0, Src1, C0, Zero, Idx, eq, select, relu
    from operator import add
    import concourse.dve_ops as dve_ops_mod
    from concourse.dve_spec import lower, _has_src1
    from concourse.dve_uop import DveOpSpec

    def _register(name, spec):
        for op in OPS:
            if op.name == name:
                return op
        row = max(dve_ops_mod._SUB_OPCODE_FOR_NAME.values()) + 1
        assert row < 0x20
        dve_ops_mod._SUB_OPCODE_FOR_NAME[name] = row
        shas = {}
        for ver in ("v3", "v4"):
            try:
                tmp = DveOpSpec(name=name, opcode=row, uops=lower(spec, ver=ver),
                                rd1_en=_has_src1(spec))
                shas[ver] = tmp.sha(ver)
            except Exception:
                pass
        op = DveOp(name, spec, subdim=False, uops_sha=shas)
        OPS.append(op)
        dve_ops_mod.CUSTOM_DVE_SPECS[name] = spec
        return op

    def _diag_ref(in0, in1, s0, s1, imm2):
        in2 = in0.reshape(in0.shape[0], -1).astype(np.float32)
        idx = np.arange(in2.shape[1])[None, :]
        sel = np.where(idx == s0, in2, 0.0).astype(np.float32)
        return sel.reshape(in0.shape), sel.sum(axis=-1, keepdims=True)

    diag_op = _register(
        "DIAG_EXTRACT_ANT",
        Spec(body=select(eq(Idx, C0), Src0, Zero), accum=add,
             accum_init=Zero, reference=_diag_ref))

    def _relu_mul_ref(in0, in1, s0, s1, imm2):
        a = np.nan_to_num(in0.astype(np.float32), nan=0.0,
                          posinf=np.inf, neginf=-np.inf)
        b = (np.maximum(a, 0.0) * in1.astype(np.float32)).astype(np.float32)
        return b, b.reshape(b.shape[0], -1).sum(axis=-1, keepdims=True)

    relu_mul_op = _register(
        "RELU_MUL_REDUCE_ANT",
        Spec(body=relu(Src0) * Src1, accum=add,
             accum_init=Zero, reference=_relu_mul_ref))

    return diag_op, relu_mul_op


def _pair_plan():
    """49 shift-pairs (1,2)..(97,98) + single shift 99.

    Returns a list of (shift_pair, kind) where kind is:
      'DD'  both halves DVE custom op from PSUM        (DVE 749/shift)
      'ADP' Act pair relu copy; DVE custom op from SBUF (Act 519 + DVE 630)
      'TT'  transposed: t^T per z-block, Act relu pair, DVE 2x TT mult,
            PE ones-window matmuls reduce over z partitions (DVE only 364)
    Cycle of 12 pairs: 8 TT, 3 ADP, 1 DD -> x_T~64, x_AD~26, x_D~9.
    """
    cycle = ["TT", "TT", "ADP", "TT", "TT", "ADP",
             "TT", "DD", "TT", "TT", "ADP", "TT"]
    plan = [(p, cycle[p % 12]) for p in range(49)]
    return plan


def _build():
    diag_op, relu_mul_op = _register_custom_ops()
    nc = bacc.Bacc("TRN2", target_bir_lowering=False, debug=False,
                   num_devices=NCORES)

    def din(name, shape, dtype=f32):
        return nc.dram_tensor(name, shape, dtype, kind="ExternalInput")

    # pre-tiled on host: [p, k, m] layouts flattened to [R, k*m]
    d_zz = din("zz_tiled", [R, 8 * R], bf16)       # zT | znT k-tiles
    d_znhT = din("znhT_tiled", [R, 4 * B], bf16)   # z_next_hat^T k-tiled
    d_WW = din("WW_tiled", [R, 8 * ZH], bf16)      # W1z | W2T k-tiles
    # packed bf16 blob: WaA | aTA | W1a | b1 | b2t | I128 | XONES
    d_blob = din("blob", [R, 1732], bf16)
    # fp8 shifted-identity blob: E2 [0,256) | E3 [256,512) | pad
    d_e8 = din("e8blob", [R, 640], fp8e4)
    # packed fp32 rows: z | z_next | dpos0 | dpos1
    d_zp = din("zpack", [R, 2 * Z + 2])

    d_out = nc.dram_tensor("out4", [R, 4], f32, kind="ExternalOutput")

    plan = _pair_plan()

    with tile.TileContext(nc) as tc:
        with tc.tile_pool(name="w", bufs=1) as wp, \
             tc.tile_pool(name="tsb", bufs=4) as tsbp, \
             tc.tile_pool(name="scrd", bufs=4) as scrdp, \
             tc.tile_pool(name="scrp", bufs=4) as scrpp, \
             tc.tile_pool(name="scr", bufs=4) as scrp, \
             tc.tile_pool(name="sm", bufs=1) as smp, \
             tc.tile_pool(name="pssim", bufs=1, space="PSUM") as ps_sim:

            # ---------------- load inputs (small/dep-critical first) -------
            _tagn = [0]

            def load(dram_ap, shape, dtype=f32, pool=wp):
                _tagn[0] += 1
                t = pool.tile(shape, dtype, tag=f"in{_tagn[0]}")
                nc.sync.dma_start(t[:], dram_ap)
                return t

            blob_t = load(d_blob[:, :], [R, 1732], bf16)
            WaA_t = blob_t[0:A + 1, 0:AH]
            aTA_t = blob_t[0:A + 1, AH:AH + 2 * R]
            W1a_t = blob_t[0:AH, 320:832]
            I128_t = blob_t[:, 1348:1476]
            e8_t = load(d_e8[:, :], [R, 640], fp8e4)
            ones_t = wp.tile([1, R], bf16, tag="ones")
            nc.gpsimd.memset(ones_t[:], 1.0)
            zz_t = load(d_zz[:, :].rearrange("p (k m) -> p k m", k=8),
                        [R, 8, R], bf16)
            WW_t = wp.tile([R, 8, ZH], bf16, tag="WWt")
            _ww = d_WW[:, :].rearrange("p (k m) -> p k m", k=8)
            nc.sync.dma_start(WW_t[:, 0:4, :], _ww[:, 0:4, :])   # W1z first (u)
            nc.sync.dma_start(WW_t[:, 4:8, :], _ww[:, 4:8, :])

            # DoubleRow lhsT windows into the fp8 blob
            def e_window(i):
                return e8_t[:, i:i + 512].rearrange(
                    "p (k m) -> p k m", k=2)[:, :, 0:R]

            # ---------------- setup compute ----------------
            g8 = wp.tile([R, 2, ZH], fp8e4, tag="g8")
            upair = wp.tile([R, 2, ZH], fp8e4, tag="upair")
            nc.gpsimd.memset(upair[:, 1, :], 0.0)
            with tc.tile_pool(name="pse", bufs=1, space="PSUM") as pse:
                # ha^T = relu(Wa_aug^T @ aT_aug): [64, 256]
                p_haT = pse.tile([AH, 2 * R], f32)
                nc.tensor.matmul(p_haT[:], WaA_t, aTA_t, start=True, stop=True)
                haT = wp.tile([AH, 2 * R], bf16)
                nc.scalar.activation(haT[:], p_haT[:], AF.Relu)

                # g tiles (fp8): g[t] = haT[:, t].T @ W1a  -> [128, 512]
                for t in range(2):
                    p_g = pse.tile([R, ZH], f32, tag="pg")
                    nc.tensor.matmul(p_g[:], haT[:, t * R:(t + 1) * R], W1a_t,
                                     start=True, stop=True)
                    nc.scalar.copy(g8[:, t, :], p_g[:])

                # v = z_next @ W2^T -> bf16
                p_v = pse.tile([R, ZH], f32)
                for k in range(4):
                    nc.tensor.matmul(p_v[:], zz_t[:, 4 + k, :], WW_t[:, 4 + k, :],
                                     start=(k == 0), stop=(k == 3))
                v_sb = wp.tile([R, ZH], bf16)
                nc.scalar.copy(v_sb[:], p_v[:])
                # v^T blocks for the transposed path: vT[z', zb*128+b]
                p_vt = pse.tile([R, ZH], bf16)
                for zb in range(4):
                    nc.tensor.matmul(p_vt[:, zb * R:(zb + 1) * R],
                                     v_sb[:, zb * R:(zb + 1) * R], I128_t,
                                     is_transpose=True, start=True, stop=True)
                vT_sb = wp.tile([R, ZH], bf16)
                nc.scalar.copy(vT_sb[:], p_vt[:])

                # u = z @ W1z + b1 -> fp8 (slot 0 of upair)
                p_u = pse.tile([R, ZH], f32)
                for k in range(4):
                    nc.tensor.matmul(p_u[:], zz_t[:, k, :], WW_t[:, k, :],
                                     start=(k == 0), stop=False)
                nc.tensor.matmul(p_u[:], ones_t[:], blob_t[0:1, 832:1344],
                                 start=False, stop=True)
                nc.scalar.copy(upair[:, 0, :], p_u[:])

                # c1 = z_next @ b2 -> [128, 1] psum
                p_c1 = pse.tile([R, 1], f32)
                for k in range(4):
                    nc.tensor.matmul(p_c1[:], zz_t[:, 4 + k, :],
                                     blob_t[:, 1344 + k:1345 + k],
                                     start=(k == 0), stop=(k == 3))
                c1_sb = smp.tile([R, 1], f32)
                nc.scalar.copy(c1_sb[:], p_c1[:])

            # ---------------- neg-sim loop ----------------
            negsim = smp.tile([R, NSH], f32)
            nc.gpsimd.memset(negsim[:], 0.0)
            sim_ps = ps_sim.tile([R, B], f32)
            znhT_t = None
            m1 = smp.tile([R, 1], f32)
            diag = smp.tile([R, 1], f32)
            Ssim = smp.tile([R, 1], f32)
            csim = smp.tile([R, 1], f32)
            nm1 = smp.tile([R, 1], f32)

            def dve_relu_mul(in0_ap, i, tag):
                scr = scrdp.tile([R, ZH], bf16, tag=tag)
                nc.vector._custom_dve(relu_mul_op, out=scr[:], in0=in0_ap,
                                      in1=v_sb[:],
                                      accum_out=negsim[:, i - 1:i])

            def pool_relu_mul(in0_ap, i, tag):
                # Pool has no STT/accum path: TT mult then TR reduce.
                scr = scrpp.tile([R, ZH], bf16, tag=tag)
                nc.gpsimd.tensor_tensor(scr[:], in0_ap, v_sb[:], op=AL.mult)
                nc.gpsimd.tensor_reduce(negsim[:, i - 1:i], scr[:],
                                        axis=mybir.AxisListType.X, op=AL.add)

            t_shifts = sorted(
                2 * p + 1 + h for p, kind in plan if kind == "TT"
                for h in (0, 1))
            first_t, last_t = t_shifts[0], t_shifts[-1]

            def xones_win(i):
                return blob_t[:, 1476 + R - i:1476 + 2 * R - i]

            with tc.tile_pool(name="pnt", bufs=1, space="PSUM") as pnt, \
                 tc.tile_pool(name="pst", bufs=2, space="PSUM") as pst:
                negT_ps = pnt.tile([R, R], f32)

                def emit_pair(shifts, kind):
                    pairp = pst.tile([R, 2, ZH], f32, tag="pairp")
                    if kind == "TT":
                        # transposed: pairp[:, h, zb*128:+128] = t^T z-block
                        for h, i in enumerate(shifts):
                            for zb in range(4):
                                sl = slice(zb * R, (zb + 1) * R)
                                nc.tensor.matmul(
                                    pairp[:, h, sl], g8[:, :, sl], e_window(i),
                                    start=True, stop=False,
                                    perf_mode=PM.DoubleRow)
                                nc.tensor.matmul(
                                    pairp[:, h, sl], upair[:, :, sl],
                                    e_window(0), start=False, stop=True,
                                    perf_mode=PM.DoubleRow)
                        tpair = tsbp.tile([R, 2, ZH], bf16, tag="tpair")
                        nc.scalar.activation(tpair[:], pairp[:], AF.Relu)
                        for h, i in enumerate(shifts):
                            prod = scrpp.tile([R, ZH], bf16, tag="prod")
                            nc.vector.tensor_tensor(prod[:], tpair[:, h, :],
                                                    vT_sb[:], op=AL.mult)
                            for zb in range(4):
                                nc.tensor.matmul(
                                    negT_ps[:], xones_win(i),
                                    prod[:, zb * R:(zb + 1) * R],
                                    start=(i == first_t and zb == 0),
                                    stop=(i == last_t and zb == 3),
                                    skip_group_check=True)
                        return
                    for h, i in enumerate(shifts):
                        nc.tensor.matmul(pairp[:, h, :], e_window(i), g8[:],
                                         start=True, stop=False,
                                         perf_mode=PM.DoubleRow)
                        nc.tensor.matmul(pairp[:, h, :], e_window(0), upair[:],
                                         start=False, stop=True,
                                         perf_mode=PM.DoubleRow)
                    if kind == "DD":
                        for h, i in enumerate(shifts):
                            dve_relu_mul(pairp[:, h, :], i, "sd")
                    else:
                        tpair = tsbp.tile([R, 2, ZH], bf16, tag="tpair")
                        if len(shifts) == 2:
                            nc.scalar.activation(tpair[:], pairp[:], AF.Relu)
                        else:
                            nc.scalar.activation(tpair[:, 0, :], pairp[:, 0, :],
                                                 AF.Relu)
                        for h, i in enumerate(shifts):
                            dve_relu_mul(tpair[:, h, :], i, "sd")

                for p, kind in plan:
                    emit_pair((2 * p + 1, 2 * p + 2), kind)

                    if p == 11:
                        # big sim input arrives while the loop runs
                        znhT_t = load(d_znhT[:, :].rearrange(
                            "p (k m) -> p k m", k=4), [R, 4, B], bf16)
                    if p == 27:
                        # sim matmuls fill PE gaps mid-loop
                        for hh in range(2):
                            for k in range(4):
                                nc.tensor.matmul(
                                    sim_ps[:, hh * 512:(hh + 1) * 512],
                                    zz_t[:, 4 + k, :],
                                    znhT_t[:, k, hh * 512:(hh + 1) * 512],
                                    start=(k == 0), stop=(k == 3))
                    if p == 17:
                        zp_t = load(d_zp[:, :], [R, 2 * Z + 2])
                        z_t = zp_t[:, 0:Z]
                        zn_t = zp_t[:, Z:2 * Z]
                        dp0_t = zp_t[:, 2 * Z:2 * Z + 1]
                        dp1_t = zp_t[:, 2 * Z + 1:2 * Z + 2]
                    if p == 31:
                        # c0 = rowdot(z, z_next); base = c0 + c1
                        c0 = smp.tile([R, 1], f32)
                        c0scr = scrp.tile([R, Z], f32, tag="scr32")
                        nc.vector.scalar_tensor_tensor(
                            out=c0scr[:], in0=z_t, scalar=0.0, in1=zn_t,
                            op0=AL.add, op1=AL.mult, accum_out=c0[:])
                        base = smp.tile([R, 1], f32)
                        nc.vector.tensor_add(base[:], c0[:], c1_sb[:])
                        # sim-side row stats overlap the loop tail
                        nc.vector.tensor_reduce(
                            m1[:], sim_ps[:], axis=mybir.AxisListType.X,
                            op=AL.max)
                        nc.vector.tensor_scalar_mul(nm1[:], m1[:], -INV_T)
                        dsc = scrp.tile([R, 512], f32, tag="scr32")
                        dA = smp.tile([R, 1], f32)
                        nc.vector._custom_dve(diag_op, out=dsc[:],
                                              accum_out=dA[:],
                                              in0=sim_ps[:, 0:512],
                                              s0=dp0_t)
                        dsc2 = scrp.tile([R, 512], f32, tag="scr32")
                        dB = smp.tile([R, 1], f32)
                        nc.vector._custom_dve(diag_op, out=dsc2[:],
                                              accum_out=dB[:],
                                              in0=sim_ps[:, 512:1024],
                                              s0=dp1_t)
                        nc.vector.tensor_add(diag[:], dA[:], dB[:])
                        sA = smp.tile([R, 1], f32, tag="sA")
                        sB = smp.tile([R, 1], f32, tag="sB")
                        for blk, sx in ((0, sA), (1, sB)):
                            eo = scrp.tile([R, 512], f32, tag="scr32")
                            nc.scalar.activation(
                                eo[:], sim_ps[:, blk * 512:(blk + 1) * 512],
                                AF.Exp, bias=nm1[:], scale=INV_T,
                                accum_out=sx[:])
                        nc.vector.tensor_add(Ssim[:], sA[:], sB[:])
                        sim_sb = wp.tile([R, B], f32, tag="simsb")
                        nc.scalar.copy(sim_sb[:, 0:512], sim_ps[:, 0:512])
                        nc.scalar.copy(sim_sb[:, 512:1024],
                                       sim_ps[:, 512:1024])
                        cA = smp.tile([R, 1], f32, tag="cA")
                        cB = smp.tile([R, 1], f32, tag="cB")
                        for blk, cx in ((0, cA), (1, cB)):
                            co = scrp.tile([R, 512], f32, tag="scr32")
                            nc.vector.tensor_scalar(
                                out=co[:],
                                in0=sim_sb[:, blk * 512:(blk + 1) * 512],
                                scalar1=diag[:], scalar2=0.0,
                                op0=AL.is_gt, op1=AL.add, accum_out=cx[:])
                        nc.vector.tensor_add(csim[:], cA[:], cB[:])

                emit_pair((NSH,), "DD")

                # pull the transposed-path accumulator out of PSUM
                negT_sb = wp.tile([R, R], bf16, tag="negTsb")
                nc.scalar.copy(negT_sb[:], negT_ps[:])

            # ---------------- tail (neg side + combine) ----------------
            with tc.tile_pool(name="ptr", bufs=1, space="PSUM") as ptr:
                ntb_ps = ptr.tile([R, R], bf16)
                nc.tensor.matmul(ntb_ps[:], negT_sb[:], I128_t,
                                 is_transpose=True, start=True, stop=True)
                nT2 = smp.tile([R, NSH], f32)
                nc.scalar.copy(nT2[:], ntb_ps[:, 1:NSH + 1])
            negsum = smp.tile([R, NSH], f32)
            nc.vector.tensor_add(negsum[:], negsim[:], nT2[:])

            negf = smp.tile([R, NSH], f32)
            nc.vector.tensor_scalar_add(negf[:], negsum[:], base[:])

            m2 = smp.tile([R, 1], f32)
            nc.vector.tensor_reduce(m2[:], negf[:], axis=mybir.AxisListType.X,
                                    op=AL.max)
            m = smp.tile([R, 1], f32)
            nc.vector.tensor_max(m[:], m1[:], m2[:])
            negb = smp.tile([R, 1], f32)
            nc.vector.tensor_scalar_mul(negb[:], m[:], -INV_T)

            # neg-side exp sum with global max
            eo = scrp.tile([R, NSH], f32, tag="scrn")
            Sneg = smp.tile([R, 1], f32)
            nc.scalar.activation(eo[:], negf[:], AF.Exp, bias=negb[:],
                                 scale=INV_T, accum_out=Sneg[:])
            # rescale sim-side sum: S = Ssim * exp((m1 - m)/T) + Sneg
            dm = smp.tile([R, 1], f32)
            nc.vector.tensor_sub(dm[:], m1[:], m[:])
            r1 = smp.tile([R, 1], f32)
            nc.scalar.activation(r1[:], dm[:], AF.Exp, scale=INV_T)
            Ssc = smp.tile([R, 1], f32)
            nc.vector.tensor_mul(Ssc[:], Ssim[:], r1[:])
            S = smp.tile([R, 1], f32)
            nc.vector.tensor_add(S[:], Ssc[:], Sneg[:])

            # neg-side counts + combine
            co = scrp.tile([R, NSH], f32, tag="scrn")
            cneg = smp.tile([R, 1], f32)
            nc.vector.tensor_scalar(out=co[:], in0=negf[:],
                                    scalar1=diag[:], scalar2=0.0,
                                    op0=AL.is_gt, op1=AL.add, accum_out=cneg[:])
            cnt = smp.tile([R, 1], f32)
            nc.vector.tensor_add(cnt[:], csim[:], cneg[:])

            out4 = smp.tile([R, 4], f32)
            nc.scalar.copy(out4[:, 0:1], m[:])
            nc.scalar.copy(out4[:, 1:2], S[:])
            nc.scalar.copy(out4[:, 2:3], diag[:])
            nc.scalar.copy(out4[:, 3:4], cnt[:])
            nc.sync.dma_start(d_out[:, :], out4[:])

    nc.compile()
    return nc


def _prepare_in_maps(z, z_next, z_next_hat, actions, Wa, ba, W1, b1, W2, b2):
    f = np.float32
    z = np.ascontiguousarray(z, f)
    z_next = np.ascontiguousarray(z_next, f)
    z_next_hat = np.ascontiguousarray(z_next_hat, f)
    actions = np.ascontiguousarray(actions, f)
    Wa = np.ascontiguousarray(Wa, f)
    ba = np.ascontiguousarray(ba, f)
    W1 = np.ascontiguousarray(W1, f)
    b1 = np.ascontiguousarray(b1, f)
    W2 = np.ascontiguousarray(W2, f)
    b2 = np.ascontiguousarray(b2, f)

    def ktile(x, km=4):
        # [k*128, M] -> [128, k*M] with [p, k, m] semantics
        k = x.shape[0] // R
        return np.ascontiguousarray(
            x.reshape(k, R, -1).transpose(1, 0, 2).reshape(R, -1))

    bf = ml_dtypes.bfloat16
    f8 = ml_dtypes.float8_e4m3
    znhT_tiled = ktile(np.ascontiguousarray(z_next_hat.T)).astype(bf)
    W1a = np.ascontiguousarray(W1[Z:]).astype(bf)
    WW_tiled = np.concatenate([ktile(np.ascontiguousarray(W1[:Z])),
                               ktile(np.ascontiguousarray(W2.T))],
                              axis=1).astype(bf)
    b2_tiled = np.ascontiguousarray(b2.reshape(4, R).T).astype(bf)
    Wa_aug = np.vstack([Wa, ba[None, :]]).astype(bf)
    b1_r = b1.reshape(1, ZH).astype(bf)

    e8 = np.zeros((R, 640), f)
    e8[:, 0:R] = np.eye(R, dtype=f)        # E2 = [I 0]
    e8[:, 256 + R:256 + 2 * R] = np.eye(R, dtype=f)  # E3 = [0 I]
    e8 = e8.astype(f8)

    in_maps = []
    for c in range(NCORES):
        s = c * R
        idx = (s + np.arange(2 * R)) % B
        a_sl = actions[idx]                      # [256, 8]
        aT_aug = np.ascontiguousarray(
            np.vstack([a_sl.T, np.ones((1, 2 * R), f)])).astype(bf)
        dpos0 = (s + np.arange(R, dtype=f)).reshape(R, 1)
        zz_tiled = np.concatenate(
            [ktile(np.ascontiguousarray(z[s:s + R].T)),
             ktile(np.ascontiguousarray(z_next[s:s + R].T))],
            axis=1).astype(bf)
        blob = np.zeros((R, 1732), bf)
        blob[0:A + 1, 0:AH] = Wa_aug
        blob[0:A + 1, AH:AH + 2 * R] = aT_aug
        blob[0:AH, 320:832] = W1a
        blob[0:1, 832:1344] = b1_r
        blob[:, 1344:1348] = b2_tiled
        blob[:, 1348:1476] = np.eye(R, dtype=f)   # I128 for transposes
        blob[:, 1476 + R] = 1.0                   # XONES: col 128 of [1476:1732)
        zpack = np.concatenate(
            [z[s:s + R], z_next[s:s + R], dpos0.astype(f),
             (dpos0 - 512.0).astype(f)], axis=1)
        in_maps.append({
            "zz_tiled": zz_tiled,
            "znhT_tiled": znhT_tiled,
            "WW_tiled": WW_tiled,
            "blob": blob,
            "e8blob": e8,
            "zpack": np.ascontiguousarray(zpack, f),
        })
    return in_maps


def _finalize(results):
    m = np.concatenate([r["out4"][:, 0] for r in results]).astype(np.float64)
    S = np.concatenate([r["out4"][:, 1] for r in results]).astype(np.float64)
    diag = np.concatenate([r["out4"][:, 2] for r in results]).astype(np.float64)
    cnt = np.concatenate([r["out4"][:, 3] for r in results]).astype(np.float64)

    lse = m * INV_T + np.log(S)
    loss = np.float32(np.mean(lse - diag * INV_T))
    accs = [np.float32(np.mean(cnt < k)) for k in TOP_K]
    return (loss, accs[0], accs[1], accs[2])


def kernel(z, z_next, z_next_hat, actions, Wa, ba, W1, b1, W2, b2,
           _trace=False, _trace_kwargs=None):
    if "nc" not in _cache:
        _cache["nc"] = _build()
    nc = _cache["nc"]
    in_maps = _prepare_in_maps(z, z_next, z_next_hat, actions,
                               Wa, ba, W1, b1, W2, b2)
    kw = {}
    if _trace:
        kw = dict(trace=True, **(_trace_kwargs or {}))
    res = run_bass_kernel_spmd(nc, in_maps, core_ids=list(range(NCORES)), **kw)
    _cache["last_results"] = res.results
    out = _finalize(res.results)
    if _trace:
        return out, res
    return out


if __name__ == "__main__":
    rng = np.random.RandomState(0)
    args = dict(
        z=rng.randn(B, Z).astype(np.float32),
        z_next=rng.randn(B, Z).astype(np.float32),
        z_next_hat=rng.randn(B, Z).astype(np.float32),
        actions=rng.randn(B, A).astype(np.float32),
        Wa=(rng.randn(A, AH) / np.sqrt(A)).astype(np.float32),
        ba=np.zeros(AH, np.float32),
        W1=(rng.randn(Z + AH, ZH) / np.sqrt(Z + AH)).astype(np.float32),
        b1=np.zeros(ZH, np.float32),
        W2=(rng.randn(ZH, Z) / np.sqrt(ZH)).astype(np.float32),
        b2=np.zeros(Z, np.float32),
    )
    print(kernel(**args))



# revision 22
# speedup vs baseline: 1.4093x; 1.4093x over previous
"""ControlCPC loss kernel for 8 Trainium2 NeuronCores (Bass/Tile), v3.

Row-sharded over the batch: core c owns rows [128c, 128c+128).

Algebraic reduction (B=1024, Z=512, A=8, AH=64, ZH=512, n_neg=100, T=0.1):
  sim[b, j] = z_next[b] . z_next_hat[j]
  u[b]      = z[b] @ W1[:Z] + b1
  g[j]      = relu(actions[j] @ Wa + ba) @ W1[Z:]
  v[b]      = z_next[b] @ W2.T
  neg[b, i] = base[b] + sum_zh relu(u[b,zh] + g[b+i,zh]) * v[b,zh]   i=1..99
  base[b]   = z[b].z_next[b] + b2.z_next[b]          (precomputed on host)
  row stats over x = [sim | neg]: m = rowmax, S = sum exp((x-m)/T),
  diag = sim[b,b] (bit-exact from PSUM), cnt = #{x > diag}.
Host: loss = mean(m/T + ln S - diag/T), acc_k = mean(cnt < k).

v3 engine plan (everything in transposed t^T[z', b] layout):
  - PE: fp8 DoubleRow matmuls produce t^T in PSUM per zb-block (213 ns/
    shift); per-shift reduction is 4 free n=1 ones-column matmuls that
    accumulate prod^T straight into negB[128, 99] PSUM (batch x shift
    layout -> no transpose at the end).
  - relu: Act pair copies f32->bf16 (519/shift) for AA pairs, Pool
    tensor_scalar_max (760/shift) for CC/EE pairs.
  - mult by v^T: DVE tensor_tensor bf16 (327/shift), Pool TT for EE
    pairs (1111/shift) to level load.
  - sim in two [128,512] PSUM halves; m1/diag/counts taken directly
    from PSUM; exp via Act with per-half max + rescale at the end.
  - v^T computed directly as W2 @ z_next^T on PE (no transposes).
"""

import sys

for _p in ("/opt/trn_rl_repo", "/opt/pypackages"):
    if _p not in sys.path:
        sys.path.insert(0, _p)

import numpy as np
import ml_dtypes

import concourse.bass as bass
import concourse.mybir as mybir
import concourse.tile as tile
from concourse import bacc
from concourse.bass_utils import run_bass_kernel_spmd

f32 = mybir.dt.float32
bf16 = mybir.dt.bfloat16
fp8e4 = mybir.dt.float8e4
AL = mybir.AluOpType
AF = mybir.ActivationFunctionType
PM = mybir.MatmulPerfMode

B, Z, A = 1024, 512, 8
AH, ZH = 64, 512
TEMP = 0.1
NCORES = 8
R = B // NCORES          # 128 rows per core
NSH = 99                 # shifts 1..99
INV_T = 1.0 / TEMP
TOP_K = (1, 3, 10)

_cache = {}


def _register_custom_ops():
    from concourse.dve_ops import DveOp, OPS
    from concourse.dve_spec import Spec, Src0, Src1, C0, Zero, Idx, eq, select, relu
    from operator import add
    import concourse.dve_ops as dve_ops_mod
    from concourse.dve_spec import lower, _has_src1
    from concourse.dve_uop import DveOpSpec

    def _register(name, spec):
        for op in OPS:
            if op.name == name:
                return op
        row = max(dve_ops_mod._SUB_OPCODE_FOR_NAME.values()) + 1
        assert row < 0x20
        dve_ops_mod._SUB_OPCODE_FOR_NAME[name] = row
        shas = {}
        for ver in ("v3", "v4"):
            try:
                tmp = DveOpSpec(name=name, opcode=row, uops=lower(spec, ver=ver),
                                rd1_en=_has_src1(spec))
                shas[ver] = tmp.sha(ver)
            except Exception:
                pass
        op = DveOp(name, spec, subdim=False, uops_sha=shas)
        OPS.append(op)
        dve_ops_mod.CUSTOM_DVE_SPECS[name] = spec
        return op

    def _diag_ref(in0, in1, s0, s1, imm2):
        in2 = in0.reshape(in0.shape[0], -1).astype(np.float32)
        idx = np.arange(in2.shape[1])[None, :]
        sel = np.where(idx == s0, in2, 0.0).astype(np.float32)
        return sel.reshape(in0.shape), sel.sum(axis=-1, keepdims=True)

    diag_op = _register(
        "DIAG_EXTRACT_ANT",
        Spec(body=select(eq(Idx, C0), Src0, Zero), accum=add,
             accum_init=Zero, reference=_diag_ref))

    def _relu_mul_ref(in0, in1, s0, s1, imm2):
        a = np.nan_to_num(in0.astype(np.float32), nan=0.0,
                          posinf=np.inf, neginf=-np.inf)
        b = (np.maximum(a, 0.0) * in1.astype(np.float32)).astype(np.float32)
        return b, b.reshape(b.shape[0], -1).sum(axis=-1, keepdims=True)

    relu_mul_op = _register(
        "RELU_MUL_REDUCE_ANT",
        Spec(body=relu(Src0) * Src1, accum=add,
             accum_init=Zero, reference=_relu_mul_ref))
    return diag_op, relu_mul_op


def _unit_plan():
    """33 units, unit u covers shifts (3u+1, 3u+2) as an Act-relu'd pair
    and shift 3u+3 as a DVE-fused single.

    Pair mults: h0 -> Pool (SBUF TT), h1 -> DVE. The single runs the
    custom RELU_MUL_REDUCE straight from PSUM with accum into negsim.
    Pool cannot read PSUM (BIR verifier), so every relu is Act or fused
    into the DVE custom op.
    """
    return [((3 * u + 1, 3 * u + 2), 3 * u + 3) for u in range(33)]


def _build():
    diag_op, relu_mul_op = _register_custom_ops()
    nc = bacc.Bacc("TRN2", target_bir_lowering=False, debug=False,
                   num_devices=NCORES)

    def din(name, shape, dtype=f32):
        return nc.dram_tensor(name, shape, dtype, kind="ExternalInput")

    # packed bf16 blob: Wa_aug | aT_aug | W1a | b1
    d_blob = din("blob", [R, 1344], bf16)
    # fp8 shifted-identity blob: E2 [0,256) | E3 [256,512) | pad
    d_e8 = din("e8blob", [R, 640], fp8e4)
    d_zz = din("zz_tiled", [R, 8 * R], bf16)       # zT | znT k-tiles
    d_W1z = din("w1z_tiled", [R, 4 * ZH], bf16)    # W1[:Z] k-tiles
    d_W2T = din("w2t_tiled", [R, 4 * ZH], bf16)    # W2.T k-tiles
    d_znhT = din("znhT_tiled", [R, 4 * B], bf16)   # z_next_hat^T k-tiles
    d_sc = din("scpack", [R, 4])                   # base | dp0 | dp1 | 0

    d_out = nc.dram_tensor("out4", [R, 4], f32, kind="ExternalOutput")
    d_early = nc.dram_tensor("early8", [R, 8], f32, kind="ExternalOutput")

    plan = _unit_plan()

    with tile.TileContext(nc) as tc:
        with tc.tile_pool(name="w", bufs=1) as wp, \
             tc.tile_pool(name="tp", bufs=6) as tpp, \
             tc.tile_pool(name="tpl", bufs=4) as tplp, \
             tc.tile_pool(name="pr", bufs=12) as prp, \
             tc.tile_pool(name="scr", bufs=6) as scrp, \
             tc.tile_pool(name="sm", bufs=1) as smp, \
             tc.tile_pool(name="ps", bufs=1, space="PSUM") as psp, \
             tc.tile_pool(name="pairp", bufs=2, space="PSUM") as pairpp, \
             tc.tile_pool(name="pairc", bufs=2, space="PSUM") as paircp, \
             tc.tile_pool(name="negb", bufs=1, space="PSUM") as negbp:

            # ---------------- DMA schedule ----------------
            # SP queue: blob, zz, W1z, W2T, znh (critical order).
            # Act queue: e8, scpack (tiny; Act.SEQ free early for compute).
            blob_t = wp.tile([R, 1344], bf16, tag="blob")
            nc.sync.dma_start(blob_t[:], d_blob[:, :])
            e8_t = wp.tile([R, 640], fp8e4, tag="e8")
            nc.sync.dma_start(e8_t[:], d_e8[:, :])
            sc_t = wp.tile([R, 4], f32, tag="sc")
            nc.scalar.dma_start(sc_t[:], d_sc[:, :])
            zz_t = wp.tile([R, 8, R], bf16, tag="zz")
            nc.sync.dma_start(zz_t[:], d_zz[:, :].rearrange("p (k m) -> p k m", k=8))
            W1z_t = wp.tile([R, 4, ZH], bf16, tag="w1z")
            nc.sync.dma_start(W1z_t[:], d_W1z[:, :].rearrange("p (k m) -> p k m", k=4))
            W2T_t = wp.tile([R, 4, ZH], bf16, tag="w2t")
            nc.sync.dma_start(W2T_t[:], d_W2T[:, :].rearrange("p (k m) -> p k m", k=4))
            znhT_t = wp.tile([R, 4, B], bf16, tag="znh")
            _zh = d_znhT[:, :].rearrange("p (k m) -> p k m", k=4)
            nc.sync.dma_start(znhT_t[:, :, 0:512], _zh[:, :, 0:512])
            nc.sync.dma_start(znhT_t[:, :, 512:1024], _zh[:, :, 512:1024])

            WaA_t = blob_t[0:A + 1, 0:AH]
            aTA_t = blob_t[0:A + 1, AH:AH + 2 * R]
            W1a_t = blob_t[0:AH, 320:832]
            b1_t = blob_t[0:1, 832:1344]
            base_t = sc_t[:, 0:1]
            dp0_t = sc_t[:, 1:2]
            dp1_t = sc_t[:, 2:3]

            def e_window(i):
                return e8_t[:, i:i + 512].rearrange(
                    "p (k m) -> p k m", k=2)[:, :, 0:R]

            # ---------------- setup compute ----------------
            ones1 = wp.tile([R, 1], bf16, tag="ones1")
            nc.gpsimd.memset(ones1[:], 1.0)
            onesr = wp.tile([1, R], bf16, tag="onesr")
            nc.gpsimd.memset(onesr[:], 1.0)

            g8 = wp.tile([R, 2, ZH], fp8e4, tag="g8")
            upair = wp.tile([R, 2, ZH], fp8e4, tag="upair")
            nc.gpsimd.memset(upair[:, 1, :], 0.0)

            # ha^T = relu(Wa_aug^T @ aT_aug): [64, 256]
            p_haT = psp.tile([AH, 2 * R], f32, tag="ps")
            nc.tensor.matmul(p_haT[:], WaA_t, aTA_t, start=True, stop=True)
            haT = wp.tile([AH, 2 * R], bf16, tag="haT")
            nc.scalar.activation(haT[:], p_haT[:], AF.Relu)

            # g tiles (fp8): g[t] = haT[:, t].T @ W1a -> [128, 512]
            for t in range(2):
                p_g = psp.tile([R, ZH], f32, tag="ps")
                nc.tensor.matmul(p_g[:], haT[:, t * R:(t + 1) * R], W1a_t,
                                 start=True, stop=True)
                nc.vector.tensor_copy(g8[:, t, :], p_g[:])

            # u = z @ W1z + b1 -> fp8 (slot 0 of upair)
            p_u = psp.tile([R, ZH], f32, tag="ps")
            for k in range(4):
                nc.tensor.matmul(p_u[:], zz_t[:, k, :], W1z_t[:, k, :],
                                 start=(k == 0), stop=False)
            nc.tensor.matmul(p_u[:], onesr[:], b1_t, start=False, stop=True)
            nc.vector.tensor_copy(upair[:, 0, :], p_u[:])

            # vT[z', zb*128+b] = sum_z W2[zb*128+z', z] * znT[z, b]
            # (uses a pairp-ring slot: the ring is idle until pair 2)
            p_vt = pairpp.tile([R, 4, R], f32, tag="pairp")
            for zb in range(4):
                for k in range(4):
                    nc.tensor.matmul(p_vt[:, zb, :],
                                     W2T_t[:, k, zb * R:(zb + 1) * R],
                                     zz_t[:, 4 + k, :],
                                     start=(k == 0), stop=(k == 3))
            vT_sb = wp.tile([R, ZH], bf16, tag="vT")
            nc.vector.tensor_copy(
                vT_sb[:].rearrange("p (k m) -> p k m", k=4), p_vt[:])

            # v in row layout [b, zh] for the fused DVE path
            p_v = paircp.tile([R, 1, ZH], f32, tag="pairc")
            for k in range(4):
                nc.tensor.matmul(p_v[:, 0, :], zz_t[:, 4 + k, :],
                                 W2T_t[:, k, :], start=(k == 0),
                                 stop=(k == 3))
            v_sb = wp.tile([R, ZH], bf16, tag="v")
            nc.vector.tensor_copy(v_sb[:], p_v[:, 0, :])
            negsim = wp.tile([R, NSH], f32, tag="negsim")
            nc.gpsimd.memset(negsim[:], 0.0)

            # ---------------- neg-sim loop ----------------
            negB = negbp.tile([R, NSH], f32)
            early8 = smp.tile([R, 8], f32, tag="early8")
            out4 = smp.tile([R, 4], f32, tag="out4")
            nc.gpsimd.memset(out4[:, 3:4], 0.0)

            sim_state = {}
            pending_acc = []
            pending_pmult = []

            def flush_acc():
                # deferred negB accumulation: by the time PE reaches these
                # n=1 matmuls the prod tiles are long done -> no PE stall
                for pr, i in pending_acc.pop(0):
                    for zb in range(4):
                        nc.tensor.matmul(
                            negB[:, i - 1:i], pr[:, zb * R:(zb + 1) * R],
                            ones1[:], start=(zb == 0), stop=(zb == 3),
                            skip_group_check=True)

            def emit_unit(pair_shifts, vshift, uidx):
                # Act-relu'd transposed pair -> 1 Pool + 1 DVE mult + PE acc
                pairp = pairpp.tile([R, 2, ZH], f32, tag="pairp")
                for h, i in enumerate(pair_shifts):
                    for zb in range(4):
                        sl = slice(zb * R, (zb + 1) * R)
                        nc.tensor.matmul(
                            pairp[:, h, sl], g8[:, :, sl], e_window(i),
                            start=True, stop=False, perf_mode=PM.DoubleRow)
                        nc.tensor.matmul(
                            pairp[:, h, sl], upair[:, :, sl], e_window(0),
                            start=False, stop=True, perf_mode=PM.DoubleRow)
                # non-transposed single -> fused DVE relu*v with accum
                if vshift is not None:
                    psing = paircp.tile([R, 1, ZH], f32, tag="pairc")
                    nc.tensor.matmul(psing[:, 0, :], e_window(vshift), g8[:],
                                     start=True, stop=False,
                                     perf_mode=PM.DoubleRow)
                    nc.tensor.matmul(psing[:, 0, :], e_window(0), upair[:],
                                     start=False, stop=True,
                                     perf_mode=PM.DoubleRow)
                tpr = tpp.tile([R, 2, ZH], bf16, tag="tpr")
                nc.scalar.activation(tpr[:], pairp[:], AF.Relu)
                if vshift is not None:
                    vscr = scrp.tile([R, ZH], bf16, tag="vscr")
                    nc.vector._custom_dve(
                        relu_mul_op, out=vscr[:], in0=psing[:, 0, :],
                        in1=v_sb[:], accum_out=negsim[:, vshift - 1:vshift])
                accs = []
                pending_pmult.append((tpr, pair_shifts[0]))
                pr = prp.tile([R, ZH], bf16, tag="prod")
                nc.vector.tensor_tensor(pr[:], tpr[:, 1, :], vT_sb[:],
                                        op=AL.mult)
                accs.append((pr, pair_shifts[1]))
                while len(pending_pmult) > 1:
                    tpx, i = pending_pmult.pop(0)
                    pr = prp.tile([R, ZH], bf16, tag="prod")
                    nc.gpsimd.tensor_tensor(pr[:], tpx[:, 0, :], vT_sb[:],
                                            op=AL.mult)
                    accs.append((pr, i))
                pending_acc.append(accs)
                if len(pending_acc) > 3:
                    flush_acc()

            def emit_sim_half(hh):
                # one ps-ring slot per half; stats from PSUM, then a bf16
                # SBUF copy so the bank frees before the count (which needs
                # the full diag = dA + dB, i.e. both halves).
                p_sim = psp.tile([R, 512], f32, tag="ps")
                for k in range(4):
                    nc.tensor.matmul(p_sim[:], zz_t[:, 4 + k, :],
                                     znhT_t[:, k, hh * 512:(hh + 1) * 512],
                                     start=(k == 0), stop=(k == 3))
                m1h = early8[:, hh:hh + 1]
                nc.vector.tensor_reduce(m1h, p_sim[:],
                                        axis=mybir.AxisListType.X, op=AL.max)
                # diag contribution (out-of-range index selects nothing)
                dsc = scrp.tile([R, 512], f32, tag="scr32")
                nc.vector._custom_dve(diag_op, out=dsc[:],
                                      accum_out=early8[:, 4 + hh:5 + hh],
                                      in0=p_sim[:],
                                      s0=(dp0_t if hh == 0 else dp1_t))
                nm1 = smp.tile([R, 1], f32, tag=f"nm1{hh}")
                nc.vector.tensor_scalar_mul(nm1[:], m1h, -INV_T)
                eo = scrp.tile([R, 512], f32, tag="scr32")
                nc.scalar.activation(eo[:], p_sim[:], AF.Exp,
                                     bias=nm1[:], scale=INV_T,
                                     accum_out=early8[:, 2 + hh:3 + hh])
                ssb = wp.tile([R, 512], bf16, tag=f"ssb{hh}")
                nc.vector.tensor_copy(ssb[:], p_sim[:])
                sim_state[f"sb{hh}"] = ssb

            def emit_sim_stats():
                st = sim_state
                diag = smp.tile([R, 1], f32, tag="diag")
                nc.vector.tensor_add(diag[:], early8[:, 4:5], early8[:, 5:6])
                for hh in range(2):
                    co = scrp.tile([R, 512], bf16, tag="scrc")
                    nc.vector.tensor_scalar(
                        out=co[:], in0=st[f"sb{hh}"][:], scalar1=diag[:],
                        scalar2=0.0, op0=AL.is_gt, op1=AL.add,
                        accum_out=early8[:, 6 + hh:7 + hh])
                thr = smp.tile([R, 1], f32, tag="thr")
                nc.vector.tensor_sub(thr[:], diag[:], base_t)
                st["thr"] = thr
                nc.sync.dma_start(d_early[:, :], early8[:])

            for uidx, (pair_shifts, vshift) in enumerate(plan):
                emit_unit(pair_shifts, vshift, uidx)
                if uidx == 8:
                    emit_sim_half(0)
                if uidx == 13:
                    emit_sim_half(1)
                if uidx == 16:
                    emit_sim_stats()
            accs = []
            while pending_pmult:
                tpx, i = pending_pmult.pop(0)
                pr = prp.tile([R, ZH], bf16, tag="prod")
                nc.gpsimd.tensor_tensor(pr[:], tpx[:, 0, :], vT_sb[:],
                                        op=AL.mult)
                accs.append((pr, i))
            if accs:
                pending_acc.append(accs)
            while pending_acc:
                flush_acc()

            # ---------------- tail ----------------
            # negf = negB(PSUM) + negsim(SBUF); m2 = rowmax fused in the
            # same op; then self-biased exp-sum + count, accums direct to
            # out4 = [m2, Sneg2, cntN, 0]. Rescaling happens on the host.
            st = sim_state
            negf = smp.tile([R, NSH], f32, tag="negf")
            nc.vector.tensor_tensor(negf[:], negB[:], negsim[:], op=AL.add)
            nc.vector.tensor_reduce(out4[:, 0:1], negf[:],
                                    axis=mybir.AxisListType.X, op=AL.max)
            nm2 = smp.tile([R, 1], f32, tag="nm2")
            nc.vector.tensor_scalar_mul(nm2[:], out4[:, 0:1], -INV_T)
            eo = scrp.tile([R, NSH], f32, tag="scrn")
            nc.scalar.activation(eo[:], negf[:], AF.Exp, bias=nm2[:],
                                 scale=INV_T, accum_out=out4[:, 1:2])
            co = scrp.tile([R, NSH], bf16, tag="scrnc")
            nc.vector.tensor_scalar(out=co[:], in0=negf[:],
                                    scalar1=st["thr"][:], scalar2=0.0,
                                    op0=AL.is_gt, op1=AL.add,
                                    accum_out=out4[:, 2:3])
            nc.sync.dma_start(d_out[:, :], out4[:])

    nc.compile()
    return nc


def _prepare_in_maps(z, z_next, z_next_hat, actions, Wa, ba, W1, b1, W2, b2):
    f = np.float32
    z = np.ascontiguousarray(z, f)
    z_next = np.ascontiguousarray(z_next, f)
    z_next_hat = np.ascontiguousarray(z_next_hat, f)
    actions = np.ascontiguousarray(actions, f)
    Wa = np.ascontiguousarray(Wa, f)
    ba = np.ascontiguousarray(ba, f)
    W1 = np.ascontiguousarray(W1, f)
    b1 = np.ascontiguousarray(b1, f)
    W2 = np.ascontiguousarray(W2, f)
    b2 = np.ascontiguousarray(b2, f)

    def ktile(x):
        # [k*128, M] -> [128, k*M] with [p, k, m] semantics
        k = x.shape[0] // R
        return np.ascontiguousarray(
            x.reshape(k, R, -1).transpose(1, 0, 2).reshape(R, -1))

    bf = ml_dtypes.bfloat16
    f8 = ml_dtypes.float8_e4m3
    znhT_tiled = ktile(np.ascontiguousarray(z_next_hat.T)).astype(bf)
    W1z_tiled = ktile(np.ascontiguousarray(W1[:Z])).astype(bf)
    W2T_tiled = ktile(np.ascontiguousarray(W2.T)).astype(bf)
    W1a = np.ascontiguousarray(W1[Z:]).astype(bf)
    Wa_aug = np.vstack([Wa, ba[None, :]]).astype(bf)
    b1_r = b1.reshape(1, ZH).astype(bf)

    e8 = np.zeros((R, 640), f)
    e8[:, 0:R] = np.eye(R, dtype=f)                  # E2 = [I 0]
    e8[:, 256 + R:256 + 2 * R] = np.eye(R, dtype=f)  # E3 = [0 I]
    e8 = e8.astype(f8)

    base_full = (z * z_next).sum(axis=1) + z_next @ b2   # [B]

    in_maps = []
    for c in range(NCORES):
        s = c * R
        idx = (s + np.arange(2 * R)) % B
        a_sl = actions[idx]                      # [256, 8]
        aT_aug = np.ascontiguousarray(
            np.vstack([a_sl.T, np.ones((1, 2 * R), f)])).astype(bf)
        dpos0 = (s + np.arange(R, dtype=f)).reshape(R, 1)
        zz_tiled = np.concatenate(
            [ktile(np.ascontiguousarray(z[s:s + R].T)),
             ktile(np.ascontiguousarray(z_next[s:s + R].T))],
            axis=1).astype(bf)
        blob = np.zeros((R, 1344), bf)
        blob[0:A + 1, 0:AH] = Wa_aug
        blob[0:A + 1, AH:AH + 2 * R] = aT_aug
        blob[0:AH, 320:832] = W1a
        blob[0:1, 832:1344] = b1_r
        scpack = np.concatenate(
            [base_full[s:s + R].reshape(R, 1).astype(f),
             dpos0.astype(f), (dpos0 - 512.0).astype(f),
             np.zeros((R, 1), f)], axis=1)
        in_maps.append({
            "blob": blob,
            "e8blob": e8,
            "zz_tiled": zz_tiled,
            "w1z_tiled": W1z_tiled,
            "w2t_tiled": W2T_tiled,
            "znhT_tiled": znhT_tiled,
            "scpack": np.ascontiguousarray(scpack, f),
        })
    return in_maps, base_full


def _finalize(results, base_full):
    def col(key, j):
        return np.concatenate([r[key][:, j] for r in results]).astype(np.float64)

    m1a, m1b = col("early8", 0), col("early8", 1)
    Sa, Sb = col("early8", 2), col("early8", 3)
    dA, dB = col("early8", 4), col("early8", 5)
    cA, cB = col("early8", 6), col("early8", 7)
    m2, Sn2, cN = col("out4", 0), col("out4", 1), col("out4", 2)

    base = base_full.astype(np.float64)
    diag = dA + dB
    mneg = m2 + base
    m = np.maximum(np.maximum(m1a, m1b), mneg)
    S = (Sa * np.exp((m1a - m) * INV_T) + Sb * np.exp((m1b - m) * INV_T)
         + Sn2 * np.exp((mneg - m) * INV_T))
    cnt = cA + cB + cN
    lse = m * INV_T + np.log(S)
    loss = np.float32(np.mean(lse - diag * INV_T))
    accs = [np.float32(np.mean(cnt < k)) for k in TOP_K]
    return (loss, accs[0], accs[1], accs[2])


def kernel(z, z_next, z_next_hat, actions, Wa, ba, W1, b1, W2, b2,
           _trace=False, _trace_kwargs=None):
    if "nc" not in _cache:
        _cache["nc"] = _build()
    nc = _cache["nc"]
    in_maps, base_full = _prepare_in_maps(z, z_next, z_next_hat, actions,
                                          Wa, ba, W1, b1, W2, b2)
    kw = {}
    if _trace:
        kw = dict(trace=True, **(_trace_kwargs or {}))
    res = run_bass_kernel_spmd(nc, in_maps, core_ids=list(range(NCORES)), **kw)
    _cache["last_results"] = res.results
    out = _finalize(res.results, base_full)
    if _trace:
        return out, res
    return out


if __name__ == "__main__":
    rng = np.random.RandomState(0)
    args = dict(
        z=rng.randn(B, Z).astype(np.float32),
        z_next=rng.randn(B, Z).astype(np.float32),
        z_next_hat=rng.randn(B, Z).astype(np.float32),
        actions=rng.randn(B, A).astype(np.float32),
        Wa=(rng.randn(A, AH) / np.sqrt(A)).astype(np.float32),
        ba=np.zeros(AH, np.float32),
        W1=(rng.randn(Z + AH, ZH) / np.sqrt(Z + AH)).astype(np.float32),
        b1=np.zeros(ZH, np.float32),
        W2=(rng.randn(ZH, Z) / np.sqrt(ZH)).astype(np.float32),
        b2=np.zeros(Z, np.float32),
    )
    print(kernel(**args))
